# revision 14
# baseline (speedup 1.0000x reference)
"""LocallyConnected2D (B=16, 32x32, CIN=COUT=64, 3x3, pad=1) on 8 TRN2 NeuronCores.

Shard the 32 output rows across 8 cores (4 rows each). Weights ride to the
device as fp8 e3m4 (scaled x32; exact-input rel err 1.45e-2 < 2e-2), x as
fp16, fp32 PSUM accumulate — the matmul moving operand is the weights, so
fp8 halves the dominant HBM stream (9.24 -> 4.62 MB/core/rep) while the
fp16 stationary x keeps full precision (bass allows mixed operand dtypes).

out[b,i,j,o] = sum_{c,k} x_pad[b, i+di, j+dj, cin] * W[o,cin,i,j,3di+dj].

Column-stationary formulation: input column c feeds the dj=2,1,0 taps of
output pixels c-1, c, c+1, so ONE matmul with stationary lhsT = x[:, c]
streams the weights of up to 3 adjacent pixels (N<=192).

Row taps on the contraction axis, all K=128. x is sent once as 3 "even"
panels V(2m) = x_pad rows (2m, 2m+1) stacked on partition halves — no row
is ever transferred twice:
  * pairs, even row r=2p:  taps (di0, di1) on V(2p),  M=16 (batch).
  * pairs, odd row r=2p+1: taps (di1, di2) on V(2p+2), M=16.
  * solos, rows (2p, 2p+1) merged: row 2p's di2 (x row 2p+2) + row 2p+1's
    di0 (x row 2p+1) via a block-diagonal stationary built ON-CHIP by DVE:
    persistent [128, 2, 512] tiles whose off-diagonal halves are zeroed
    once at program start; per rep two partition-aligned DVE copies refresh
    the diagonal blocks (V(2p+2)[0:64] and V(2p)[64:128]) — zero DMA-engine
    traffic. lhsT = bd[:, :, c*16:+16] (free dims 2x16 -> M=32).
    Matmul outputs land only at psum partition offsets = 0 mod 32, so the
    merged-solo results go to separate psum banks / output stream and the
    host adds the two streams.

Per rep per core: 152 pair + 76 solo matmuls, 36.1k PE streaming cycles
(the K=128 floor for 4.62M weight elements); ~5.5 MB total DMA traffic.

Host layouts (per core C, i = 4C+r, strip s = j//8, f = j%8; flat =
matmul schedule order):
  w_pairs [4, 128, 6016] f8: [64h+cin, flat]: h=0/1 = taps (dj,3+dj) for
          even rows, (3+dj,6+dj) for odd rows; = 32*W[o,cin,i,p,.]
  w_solo  [2, 128, 6016] f8: [64h+cin, flat]: h=0 = row 2p tap 6+dj,
          h=1 = row 2p+1 tap dj
  xt      [384, 512] f16:    [l*64+cin, j*16+b] = x_pad[b, 4C+l, j, cin]
  out     [4, 16, 2048] f16:  [s, b, r*512+f*64+o]      = 32*pair_part
  out2    [2, 128, 512] f16:  [p, 32*s+16*rr+b, f*64+o] = 32*solo_part

PSUM: per rep 4 pair banks + 2 solo banks [128, 512] f32, DVE-zeroed (all
matmuls accumulate with start=False; hardware start=True resets the whole
32-partition tile group, measured slower than DVE memsets);
full-partition DVE casts f32->f16 into stage tiles; output DMAs split
SP/ACT; weight DMAs split SP/ACT (pairs) + gpsimd SWDGE (solos, out2).

n_reps unrolled reps sit inside an optional tc.For_i(0, loop_iters) device
loop so the bench can run thousands of reps per dispatch (the axon tunnel
has ~100ms dispatch jitter; on-device looping makes the timing signal
dominate it).
"""

import numpy as np
import ml_dtypes

B, IH, IW, CIN = 16, 32, 32, 64
COUT, OH, OW = 64, 32, 32
NCORES, RPC = 8, 4
W_SCALE = 32.0

_NC = None


def _schedule():
    """Matmul schedule: flat list of (c, s, p_lo, npix) pieces, column-
    major over q-interleaved strips (c = 8*st + q) so consecutive matmuls
    hit different PE column groups and overlapping accumulate regions are
    several instructions apart. Shared by the kernel builder and the host
    weight packer so the flat weight layout matches consumption order."""

    def pieces_of(c):
        pixels = [p for p in (c - 1, c, c + 1) if 0 <= p < 32]
        out, run = [], []
        for p in pixels:
            if run and (p // 8 != run[0] // 8):
                out.append((c, run[0] // 8, run[0], len(run)))
                run = []
            run.append(p)
        if run:
            out.append((c, run[0] // 8, run[0], len(run)))
        return out

    ordered = []
    for q in range(8):
        for st in range(4):
            ordered.extend(pieces_of(8 * st + q))
    return ordered


def _build_nc(n_reps=1, loop_iters=1):
    import concourse.bacc as bacc
    import concourse.mybir as mybir
    import concourse.tile as tile

    f16 = mybir.dt.float16
    f32 = mybir.dt.float32
    f8 = mybir.dt.float8e3
    pieces = _schedule()
    ntap = sum(npix for _, _, _, npix in pieces)  # 94
    nmm = len(pieces)
    nc = bacc.Bacc("TRN2", target_bir_lowering=False, debug=False)
    wp = nc.dram_tensor("w_pairs", [RPC, 128, ntap * 64], f8, kind="ExternalInput")
    wso = nc.dram_tensor("w_solo", [2, 128, ntap * 64], f8, kind="ExternalInput")
    xt = nc.dram_tensor("xt", [384, 512], f16, kind="ExternalInput")
    out = nc.dram_tensor("out", [4, 16, RPC * 512], f16, kind="ExternalOutput")
    out2 = nc.dram_tensor("out2", [2, 128, 512], f16, kind="ExternalOutput")
    wp_ap, wso_ap, xt_ap = wp.ap(), wso.ap(), xt.ap()
    out_ap, out2_ap = out.ap(), out2.ap()

    # flat column offsets per piece, in schedule order
    offs = np.cumsum([0] + [npix * 64 for _, _, _, npix in pieces])

    with tile.TileContext(nc) as tc:
        with (
            tc.tile_pool(name="wp", bufs=3) as wp_pool,
            tc.tile_pool(name="wso", bufs=2) as wso_pool,
            tc.tile_pool(name="vx", bufs=2) as vx_pool,
            tc.tile_pool(name="bd", bufs=2) as bd_pool,
            tc.tile_pool(name="stage", bufs=2) as stage_pool,
            tc.tile_pool(name="psum_p", bufs=6, space="PSUM") as psum_p_pool,
            tc.tile_pool(name="psum_s", bufs=2, space="PSUM") as psum_s_pool,
        ):

            def emit_pairs(r, vs, stage):
                wp_t = wp_pool.tile([128, ntap * 64], f8, tag="wp")
                weng = nc.sync if r < 2 else nc.scalar
                weng.dma_start(wp_t[:], wp_ap[r][:])
                ps = psum_p_pool.tile([128, 512], f32, tag="psp")
                nc.vector.memset(ps[:], 0.0)
                lhs = vs[r // 2] if r % 2 == 0 else vs[r // 2 + 1]
                for mi, (c, s, p_lo, npix) in enumerate(pieces):
                    n = npix * 64
                    po = int(offs[mi])
                    pslice = ps[
                        32 * s : 32 * s + 16,
                        (p_lo % 8) * 64 : (p_lo % 8) * 64 + n,
                    ]
                    nc.tensor.matmul(
                        pslice,
                        lhs[:, c * 16 : (c + 1) * 16],
                        wp_t[:, po : po + n],
                        start=False,
                        stop=(mi == nmm - 1),
                        tile_position=(0, 32 * s),
                        skip_group_check=True,
                    )
                # Full-partition cast: unused lanes carry memset zeros, so
                # one [128, 512] copy replaces 4 [16, 512]s.
                nc.vector.tensor_copy(stage[:, r * 512 : (r + 1) * 512], ps[:, :])
                if r % 2 == 1:
                    for s in range(4):
                        oeng = nc.sync if s < 2 else nc.scalar
                        oeng.dma_start(
                            out_ap[s][:, (r - 1) * 512 : (r + 1) * 512],
                            stage[
                                32 * s : 32 * s + 16,
                                (r - 1) * 512 : (r + 1) * 512,
                            ],
                        )

            def emit_solo(p, bd, stage2):
                wso_t = wso_pool.tile([128, ntap * 64], f8, tag="wso")
                nc.gpsimd.dma_start(wso_t[:], wso_ap[p][:])
                ps = psum_s_pool.tile([128, 512], f32, tag="pss")
                nc.vector.memset(ps[:], 0.0)
                for mi, (c, s, p_lo, npix) in enumerate(pieces):
                    n = npix * 64
                    po = int(offs[mi])
                    pslice = ps[
                        32 * s : 32 * s + 32,
                        (p_lo % 8) * 64 : (p_lo % 8) * 64 + n,
                    ]
                    nc.tensor.matmul(
                        pslice,
                        bd[:, c, :],
                        wso_t[:, po : po + n],
                        start=False,
                        stop=(mi == nmm - 1),
                        tile_position=(0, 32 * s),
                        skip_group_check=True,
                    )
                nc.vector.tensor_copy(stage2[:, p * 512 : (p + 1) * 512], ps[:, :])
                nc.gpsimd.dma_start(
                    out2_ap[p][:], stage2[:, p * 512 : (p + 1) * 512]
                )

            def body(rep):
                # x once from HBM: the 3 even panels ARE the 6 padded rows.
                vs = []
                for m in range(3):
                    v = vx_pool.tile([128, 512], f16, tag=f"v{m}")
                    vs.append(v)
                    eng = nc.sync if m < 2 else nc.scalar
                    eng.dma_start(v[:], xt_ap[128 * m : 128 * m + 128])
                # build block-diagonal solo stationaries (DVE, no DMA):
                # zero the tile, then two partition-aligned diagonal copies
                bds = []
                for p in range(2):
                    bd = bd_pool.tile([128, 32, 32], f16, tag=f"bd{p}")
                    bds.append(bd)
                    nc.vector.memset(bd[:], 0.0)
                    nc.vector.tensor_copy(bd[0:64, :, 0:16], vs[p + 1][0:64, :])
                    nc.vector.tensor_copy(bd[64:128, :, 16:32], vs[p][64:128, :])

                stage = stage_pool.tile([128, 2048], f16, tag="stage")
                stage2 = stage_pool.tile([128, 1024], f16, tag="stage2")
                emit_pairs(0, vs, stage)
                emit_pairs(1, vs, stage)
                emit_solo(0, bds[0], stage2)
                emit_pairs(2, vs, stage)
                emit_pairs(3, vs, stage)
                emit_solo(1, bds[1], stage2)

            if loop_iters > 1:
                with tc.For_i(0, loop_iters):
                    for rep in range(n_reps):
                        body(rep)
            else:
                for rep in range(n_reps):
                    body(rep)
    nc.compile()
    return nc


def _repack_inputs(x, weight):
    x = np.asarray(x, dtype=np.float32)
    weight = np.asarray(weight, dtype=np.float32)
    pieces = _schedule()
    ntap = sum(npix for _, _, _, npix in pieces)

    # wt[i, cin, o, j, k]
    wt = np.ascontiguousarray(weight.transpose(2, 1, 0, 3, 4)) * W_SCALE
    wpair = np.zeros((OH, 128, ntap * 64), dtype=np.float32)
    wsolo = np.zeros((OH // 2, 128, ntap * 64), dtype=np.float32)
    off = 0
    for c, s, p_lo, npix in pieces:
        for e, p in enumerate(range(p_lo, p_lo + npix)):
            dj = c - p + 1
            pb = slice(off + 64 * e, off + 64 * (e + 1))
            # even rows: pair taps (di0, di1); odd rows: (di1, di2)
            wpair[0::2, 0:64, pb] = wt[0::2, :, :, p, dj]
            wpair[0::2, 64:128, pb] = wt[0::2, :, :, p, 3 + dj]
            wpair[1::2, 0:64, pb] = wt[1::2, :, :, p, 3 + dj]
            wpair[1::2, 64:128, pb] = wt[1::2, :, :, p, 6 + dj]
            # solo: even row di2 on top half, odd row di0 on bottom half
            wsolo[:, 0:64, pb] = wt[0::2, :, :, p, 6 + dj]
            wsolo[:, 64:128, pb] = wt[1::2, :, :, p, dj]
        off += 64 * npix
    wpair = wpair.astype(ml_dtypes.float8_e3m4)
    wsolo = wsolo.astype(ml_dtypes.float8_e3m4)

    xpad = np.zeros((IH + 2, CIN, IW, B), dtype=np.float16)
    xpad[1:33] = x.transpose(1, 3, 2, 0)  # [ih, c, j, b]

    in_maps = []
    for c in range(NCORES):
        in_maps.append(
            {
                "w_pairs": np.ascontiguousarray(wpair[c * RPC : (c + 1) * RPC]),
                "w_solo": np.ascontiguousarray(wsolo[2 * c : 2 * c + 2]),
                "xt": np.ascontiguousarray(
                    xpad[c * RPC : c * RPC + RPC + 2].reshape(384, 512)
                ),
            }
        )
    return in_maps


def _get_nc():
    global _NC
    if _NC is None:
        _NC = _build_nc()
    return _NC


def run_spmd(in_maps, **kwargs):
    from concourse.bass_utils import run_bass_kernel_spmd

    return run_bass_kernel_spmd(
        _get_nc(), in_maps, core_ids=list(range(NCORES)), **kwargs
    )


def kernel(x, weight, bias, _results=None):
    if _results is None:
        _results = run_spmd(_repack_inputs(x, weight)).results
    arr = np.stack([r["out"] for r in _results]).astype(np.float32)
    arr = arr.reshape(NCORES, 4, 16, RPC, 8, 64)
    # arr: [core, s, b, r, f, o] -> out[b, 4*core+r, 8s+f, o]
    out = arr.transpose(2, 0, 3, 1, 4, 5).reshape(B, OH, OW, COUT)
    # solo stream: [core, p, 32s+16rr+b, f*64+o] -> out[b, 4*core+2p+rr, 8s+f, o]
    arr2 = np.stack([r["out2"] for r in _results]).astype(np.float32)
    arr2 = arr2.reshape(NCORES, 2, 4, 2, 16, 8, 64)
    out2 = arr2.transpose(4, 0, 1, 3, 2, 5, 6).reshape(B, OH, OW, COUT)
    return (out + out2) / W_SCALE + np.asarray(bias, dtype=np.float32)[None]


# revision 18
# speedup vs baseline: 1.0141x; 1.0141x over previous
"""LocallyConnected2D (B=16, 32x32, CIN=COUT=64, 3x3, pad=1) on 8 TRN2 NeuronCores.

Shard the 32 output rows across 8 cores (4 rows each). Weights ride to the
device as fp8 e3m4 (scaled x32; exact-input rel err 1.45e-2 < 2e-2), x as
fp16, fp32 PSUM accumulate — the matmul moving operand is the weights, so
fp8 halves the dominant HBM stream (9.24 -> 4.62 MB/core/rep) while the
fp16 stationary x keeps full precision (bass allows mixed operand dtypes).

out[b,i,j,o] = sum_{c,k} x_pad[b, i+di, j+dj, cin] * W[o,cin,i,j,3di+dj].

Column-stationary formulation: input column c feeds the dj=2,1,0 taps of
output pixels c-1, c, c+1, so ONE matmul with stationary lhsT = x[:, c]
streams the weights of up to 3 adjacent pixels (N<=192).

Row taps on the contraction axis, all K=128. x is sent once as 3 "even"
panels V(2m) = x_pad rows (2m, 2m+1) stacked on partition halves — no row
is ever transferred twice:
  * pairs, even row r=2p:  taps (di0, di1) on V(2p),  M=16 (batch).
  * pairs, odd row r=2p+1: taps (di1, di2) on V(2p+2), M=16.
  * solos, rows (2p, 2p+1) merged: row 2p's di2 (x row 2p+2) + row 2p+1's
    di0 (x row 2p+1) via a block-diagonal stationary built ON-CHIP by DVE:
    persistent [128, 2, 512] tiles whose off-diagonal halves are zeroed
    once at program start; per rep two partition-aligned DVE copies refresh
    the diagonal blocks (V(2p+2)[0:64] and V(2p)[64:128]) — zero DMA-engine
    traffic. lhsT = bd[:, :, c*16:+16] (free dims 2x16 -> M=32).
    Matmul outputs land only at psum partition offsets = 0 mod 32, so the
    merged-solo results go to separate psum banks / output stream and the
    host adds the two streams.

Per rep per core: 152 pair + 76 solo matmuls, 36.1k PE streaming cycles
(the K=128 floor for 4.62M weight elements); ~5.5 MB total DMA traffic.

Host layouts (per core C, i = 4C+r, strip s = j//8, f = j%8; flat =
matmul schedule order):
  w_pairs [4, 128, 6016] f8: [64h+cin, flat]: h=0/1 = taps (dj,3+dj) for
          even rows, (3+dj,6+dj) for odd rows; = 32*W[o,cin,i,p,.]
  w_solo  [2, 128, 6016] f8: [64h+cin, flat]: h=0 = row 2p tap 6+dj,
          h=1 = row 2p+1 tap dj
  xt      [384, 512] f16:    [l*64+cin, j*16+b] = x_pad[b, 4C+l, j, cin]
  out     [4, 16, 2048] f16:  [s, b, r*512+f*64+o]      = 32*pair_part
  out2    [2, 128, 512] f16:  [p, 32*s+16*rr+b, f*64+o] = 32*solo_part

PSUM: per rep 4 pair banks + 2 solo banks [128, 512] f32, DVE-zeroed (all
matmuls accumulate with start=False; hardware start=True resets the whole
32-partition tile group, measured slower than DVE memsets);
full-partition DVE casts f32->f16 into stage tiles; output DMAs split
SP/ACT; weight DMAs split SP/ACT (pairs) + gpsimd SWDGE (solos, out2).

n_reps unrolled reps sit inside an optional tc.For_i(0, loop_iters) device
loop so the bench can run thousands of reps per dispatch (the axon tunnel
has ~100ms dispatch jitter; on-device looping makes the timing signal
dominate it).
"""

import numpy as np
import ml_dtypes

B, IH, IW, CIN = 16, 32, 32, 64
COUT, OH, OW = 64, 32, 32
NCORES, RPC = 8, 4
W_SCALE = 32.0

_NC = None


def _schedule():
    """Matmul schedule: flat list of (c, s, p_lo, npix) pieces, column-
    major over q-interleaved strips (c = 8*st + q) so consecutive matmuls
    hit different PE column groups and overlapping accumulate regions are
    several instructions apart. Shared by the kernel builder and the host
    weight packer so the flat weight layout matches consumption order."""

    def pieces_of(c):
        pixels = [p for p in (c - 1, c, c + 1) if 0 <= p < 32]
        out, run = [], []
        for p in pixels:
            if run and (p // 8 != run[0] // 8):
                out.append((c, run[0] // 8, run[0], len(run)))
                run = []
            run.append(p)
        if run:
            out.append((c, run[0] // 8, run[0], len(run)))
        return out

    ordered = []
    for q in range(8):
        for st in range(4):
            ordered.extend(pieces_of(8 * st + q))
    return ordered


def _build_nc(n_reps=1, loop_iters=1):
    import concourse.bacc as bacc
    import concourse.mybir as mybir
    import concourse.tile as tile

    f16 = mybir.dt.float16
    f32 = mybir.dt.float32
    f8 = mybir.dt.float8e3
    pieces = _schedule()
    ntap = sum(npix for _, _, _, npix in pieces)  # 94
    nmm = len(pieces)
    nc = bacc.Bacc("TRN2", target_bir_lowering=False, debug=False)
    wp = nc.dram_tensor("w_pairs", [RPC, 128, ntap * 64], f8, kind="ExternalInput")
    wso = nc.dram_tensor("w_solo", [2, 128, ntap * 64], f8, kind="ExternalInput")
    xt = nc.dram_tensor("xt", [384, 512], f16, kind="ExternalInput")
    out = nc.dram_tensor("out", [4, 16, RPC * 512], f16, kind="ExternalOutput")
    out2 = nc.dram_tensor("out2", [2, 128, 512], f16, kind="ExternalOutput")
    wp_ap, wso_ap, xt_ap = wp.ap(), wso.ap(), xt.ap()
    out_ap, out2_ap = out.ap(), out2.ap()

    # flat column offsets per piece, in schedule order
    offs = np.cumsum([0] + [npix * 64 for _, _, _, npix in pieces])

    with tile.TileContext(nc) as tc:
        with (
            tc.tile_pool(name="wp", bufs=3) as wp_pool,
            tc.tile_pool(name="wso", bufs=2) as wso_pool,
            tc.tile_pool(name="vx", bufs=2) as vx_pool,
            tc.tile_pool(name="bd", bufs=2) as bd_pool,
            tc.tile_pool(name="stage", bufs=2) as stage_pool,
            tc.tile_pool(name="psum_p", bufs=6, space="PSUM") as psum_p_pool,
            tc.tile_pool(name="psum_s", bufs=2, space="PSUM") as psum_s_pool,
        ):

            def emit_pairs(r, vs, stage, ps):
                wp_t = wp_pool.tile([128, ntap * 64], f8, tag="wp")
                weng = nc.sync if r < 2 else nc.scalar
                weng.dma_start(wp_t[:], wp_ap[r][:])
                lhs = vs[r // 2] if r % 2 == 0 else vs[r // 2 + 1]
                for mi, (c, s, p_lo, npix) in enumerate(pieces):
                    n = npix * 64
                    po = int(offs[mi])
                    pslice = ps[
                        32 * s : 32 * s + 16,
                        (p_lo % 8) * 64 : (p_lo % 8) * 64 + n,
                    ]
                    nc.tensor.matmul(
                        pslice,
                        lhs[:, c * 16 : (c + 1) * 16],
                        wp_t[:, po : po + n],
                        start=False,
                        stop=(mi == nmm - 1),
                        tile_position=(0, 32 * s),
                        skip_group_check=True,
                    )
                # Full-partition cast: unused lanes carry memset zeros, so
                # one [128, 512] copy replaces 4 [16, 512]s.
                nc.vector.tensor_copy(stage[:, r * 512 : (r + 1) * 512], ps[:, :])
                if r % 2 == 1:
                    for s in range(4):
                        oeng = nc.sync if s < 2 else nc.scalar
                        oeng.dma_start(
                            out_ap[s][:, (r - 1) * 512 : (r + 1) * 512],
                            stage[
                                32 * s : 32 * s + 16,
                                (r - 1) * 512 : (r + 1) * 512,
                            ],
                        )

            def emit_solo(p, bd, stage2, ps, last):
                wso_t = wso_pool.tile([128, ntap * 64], f8, tag="wso")
                nc.gpsimd.dma_start(wso_t[:], wso_ap[p][:])
                for mi, (c, s, p_lo, npix) in enumerate(pieces):
                    n = npix * 64
                    po = int(offs[mi])
                    pslice = ps[
                        32 * s : 32 * s + 32,
                        (p_lo % 8) * 64 : (p_lo % 8) * 64 + n,
                    ]
                    nc.tensor.matmul(
                        pslice,
                        bd[:, c, :],
                        wso_t[:, po : po + n],
                        start=False,
                        stop=(mi == nmm - 1),
                        tile_position=(0, 32 * s),
                        skip_group_check=True,
                    )
                if last:
                    nc.scalar.copy(stage2[:, p * 512 : (p + 1) * 512], ps[:, :])
                else:
                    nc.vector.tensor_copy(
                        stage2[:, p * 512 : (p + 1) * 512], ps[:, :]
                    )
                nc.scalar.dma_start(
                    out2_ap[p][:], stage2[:, p * 512 : (p + 1) * 512]
                )

            def body(rep):
                # x once from HBM: the 3 even panels ARE the 6 padded rows.
                vs = []
                for m in range(3):
                    v = vx_pool.tile([128, 512], f16, tag=f"v{m}")
                    vs.append(v)
                    eng = nc.sync if m < 2 else nc.scalar
                    eng.dma_start(v[:], xt_ap[128 * m : 128 * m + 128])
                # all psum banks + memsets up-front on DVE (no matmul
                # waits), then bd builds; DVE casts come after, so no
                # engine-stream head-of-line blocking stalls the PE
                psp = [
                    psum_p_pool.tile([128, 512], f32, tag="psp", name=f"psp{i}")
                    for i in range(4)
                ]
                pss = [
                    psum_s_pool.tile([128, 512], f32, tag="pss", name=f"pss{i}")
                    for i in range(2)
                ]
                for t in psp + pss:
                    nc.vector.memset(t[:], 0.0)
                # block-diagonal solo stationaries (DVE, no DMA traffic):
                # zero the tile, then two partition-aligned diagonal copies
                bds = []
                for p in range(2):
                    bd = bd_pool.tile([128, 32, 32], f16, tag=f"bd{p}")
                    bds.append(bd)
                    nc.vector.memset(bd[:], 0.0)
                    nc.vector.tensor_copy(bd[0:64, :, 0:16], vs[p + 1][0:64, :])
                    nc.vector.tensor_copy(bd[64:128, :, 16:32], vs[p][64:128, :])

                stage = stage_pool.tile([128, 2048], f16, tag="stage")
                stage2 = stage_pool.tile([128, 1024], f16, tag="stage2")
                emit_pairs(0, vs, stage, psp[0])
                emit_pairs(1, vs, stage, psp[1])
                emit_solo(0, bds[0], stage2, pss[0], last=False)
                emit_pairs(2, vs, stage, psp[2])
                emit_pairs(3, vs, stage, psp[3])
                emit_solo(1, bds[1], stage2, pss[1], last=True)

            if loop_iters > 1:
                with tc.For_i(0, loop_iters):
                    for rep in range(n_reps):
                        body(rep)
            else:
                for rep in range(n_reps):
                    body(rep)
    nc.compile()
    return nc


def _repack_inputs(x, weight):
    x = np.asarray(x, dtype=np.float32)
    weight = np.asarray(weight, dtype=np.float32)
    pieces = _schedule()
    ntap = sum(npix for _, _, _, npix in pieces)

    # wt[i, cin, o, j, k]
    wt = np.ascontiguousarray(weight.transpose(2, 1, 0, 3, 4)) * W_SCALE
    wpair = np.zeros((OH, 128, ntap * 64), dtype=np.float32)
    wsolo = np.zeros((OH // 2, 128, ntap * 64), dtype=np.float32)
    off = 0
    for c, s, p_lo, npix in pieces:
        for e, p in enumerate(range(p_lo, p_lo + npix)):
            dj = c - p + 1
            pb = slice(off + 64 * e, off + 64 * (e + 1))
            # even rows: pair taps (di0, di1); odd rows: (di1, di2)
            wpair[0::2, 0:64, pb] = wt[0::2, :, :, p, dj]
            wpair[0::2, 64:128, pb] = wt[0::2, :, :, p, 3 + dj]
            wpair[1::2, 0:64, pb] = wt[1::2, :, :, p, 3 + dj]
            wpair[1::2, 64:128, pb] = wt[1::2, :, :, p, 6 + dj]
            # solo: even row di2 on top half, odd row di0 on bottom half
            wsolo[:, 0:64, pb] = wt[0::2, :, :, p, 6 + dj]
            wsolo[:, 64:128, pb] = wt[1::2, :, :, p, dj]
        off += 64 * npix
    wpair = wpair.astype(ml_dtypes.float8_e3m4)
    wsolo = wsolo.astype(ml_dtypes.float8_e3m4)

    xpad = np.zeros((IH + 2, CIN, IW, B), dtype=np.float16)
    xpad[1:33] = x.transpose(1, 3, 2, 0)  # [ih, c, j, b]

    in_maps = []
    for c in range(NCORES):
        in_maps.append(
            {
                "w_pairs": np.ascontiguousarray(wpair[c * RPC : (c + 1) * RPC]),
                "w_solo": np.ascontiguousarray(wsolo[2 * c : 2 * c + 2]),
                "xt": np.ascontiguousarray(
                    xpad[c * RPC : c * RPC + RPC + 2].reshape(384, 512)
                ),
            }
        )
    return in_maps


def _get_nc():
    global _NC
    if _NC is None:
        _NC = _build_nc()
    return _NC


def run_spmd(in_maps, **kwargs):
    from concourse.bass_utils import run_bass_kernel_spmd

    return run_bass_kernel_spmd(
        _get_nc(), in_maps, core_ids=list(range(NCORES)), **kwargs
    )


def kernel(x, weight, bias, _results=None):
    if _results is None:
        _results = run_spmd(_repack_inputs(x, weight)).results
    arr = np.stack([r["out"] for r in _results]).astype(np.float32)
    arr = arr.reshape(NCORES, 4, 16, RPC, 8, 64)
    # arr: [core, s, b, r, f, o] -> out[b, 4*core+r, 8s+f, o]
    out = arr.transpose(2, 0, 3, 1, 4, 5).reshape(B, OH, OW, COUT)
    # solo stream: [core, p, 32s+16rr+b, f*64+o] -> out[b, 4*core+2p+rr, 8s+f, o]
    arr2 = np.stack([r["out2"] for r in _results]).astype(np.float32)
    arr2 = arr2.reshape(NCORES, 2, 4, 2, 16, 8, 64)
    out2 = arr2.transpose(4, 0, 1, 3, 2, 5, 6).reshape(B, OH, OW, COUT)
    return (out + out2) / W_SCALE + np.asarray(bias, dtype=np.float32)[None]
